# revision 14
# baseline (speedup 1.0000x reference)
"""DeformGAT (4-layer) Trainium2 kernel — 8 NeuronCores SPMD.

Sharding: nodes in 8 contiguous blocks of 1250 (padded to 1280); edges are
assigned to their dst node's core (edges pre-sorted by dst on host). Weights
replicated. Per layer each core gathers src rows of the replicated bf16
feature table (dma_gather), computes per-edge softmax (score broadcasts via
dst-indexed gathers + one-hot segment-sum matmuls), aggregates with bf16
scatter matmuls, applies the per-head output transform (head-mean, bias and
coordinate displacement folded into an augmented weight matrix), then
AllGathers its produced rows so every core again holds the full table.
"""
import numpy as np
import ml_dtypes
from contextlib import ExitStack

import concourse.bacc as bacc
import concourse.bass as bass
import concourse.tile as tile
import concourse.mybir as mybir
from concourse import library_config
from concourse.bass_utils import run_bass_kernel_spmd

F32 = mybir.dt.float32
BF16 = mybir.dt.bfloat16
I16 = mybir.dt.int16
AF = mybir.ActivationFunctionType
BF = ml_dtypes.bfloat16

NCORES = 8
N = 10000
E = 60000
H = 6
NL = 1250          # real nodes per core
NLP = 1280         # padded nodes per core
NBLK = 80          # dst blocks of 16 per core
NSLAB = 10         # slabs of 128 dst nodes (8 blocks)
CAP = 128          # edge capacity per block (= chunk)

# GAT layer dims (din, C). Stage s (2..5) runs GAT s-1.
GAT_DIMS = [(256, 508), (512, 250), (256, 120), (128, 20)]
FDIM = [256, 512, 256, 128]          # feat_s dim produced by stage s
ROWB = [384, 640, 384, 256]          # bf16 row elems (din + 6 scores, %128)

SELU_L = 1.0507009873554805
SELU_A = 1.6732632423543772


def _build_nc():
    nc = bacc.Bacc("TRN2", target_bir_lowering=False, debug=False,
                   num_devices=NCORES)
    # ---------------- inputs ----------------
    inp = {}
    inp["dataT"] = nc.dram_tensor("dataT", [16, NLP], F32, kind="ExternalInput")
    inp["coords_loc"] = nc.dram_tensor("coords_loc", [NLP, 2], F32, kind="ExternalInput")
    inp["cfac"] = nc.dram_tensor("cfac", [NLP, 1], F32, kind="ExternalInput")
    inp["srcidx"] = nc.dram_tensor("srcidx", [128, NBLK * 8], I16, kind="ExternalInput")
    inp["p0"] = nc.dram_tensor("p0", [128, NBLK * 16], F32, kind="ExternalInput")
    inp["p0t"] = nc.dram_tensor("p0t", [16, NBLK * 128], F32, kind="ExternalInput")
    inp["p0rep"] = nc.dram_tensor("p0rep", [128, NBLK * 96], BF16, kind="ExternalInput")
    inp["ident"] = nc.dram_tensor("ident", [128, 128], BF16, kind="ExternalInput")
    inp["linW"] = nc.dram_tensor("linW", [16, 254], F32, kind="ExternalInput")
    inp["bias1"] = nc.dram_tensor("bias1", [128, 254], F32, kind="ExternalInput")
    for i in range(1, 5):
        din, C = GAT_DIMS[i - 1]
        kt = din // 128
        CP = 2 if i == 4 else C + 2
        inp[f"wp{i}"] = nc.dram_tensor(f"wp{i}", [128, kt * 6 * CP], BF16, kind="ExternalInput")
        if i < 4:
            inp[f"biasg{i}"] = nc.dram_tensor(f"biasg{i}", [128, C], F32, kind="ExternalInput")
        ktf = FDIM[i - 1] // 128
        inp[f"wsc{i}"] = nc.dram_tensor(f"wsc{i}", [128, ktf * 12], BF16, kind="ExternalInput")
    out_t = nc.dram_tensor("out", [NLP, 2], F32, kind="ExternalOutput")

    rg = [list(range(NCORES))]

    with tile.TileContext(nc) as tc, ExitStack() as ctx:
        persist = ctx.enter_context(tc.tile_pool(name="persist", bufs=1))
        dram = ctx.enter_context(tc.tile_pool(name="dram", bufs=1, space="DRAM"))
        fg_pool = ctx.enter_context(tc.tile_pool(name="fg", bufs=3))
        gt_pool = ctx.enter_context(tc.tile_pool(name="gt", bufs=2))
        m_pool = ctx.enter_context(tc.tile_pool(name="m", bufs=2))
        e_pool = ctx.enter_context(tc.tile_pool(name="ep", bufs=2))
        fn_pool = ctx.enter_context(tc.tile_pool(name="fn", bufs=2))
        fnt_pool = ctx.enter_context(tc.tile_pool(name="fnt", bufs=2))
        wp_pool = ctx.enter_context(tc.tile_pool(name="wp", bufs=2))
        p0t_pool = ctx.enter_context(tc.tile_pool(name="p0tp", bufs=2))
        small = ctx.enter_context(tc.tile_pool(name="small", bufs=3))
        ps_gt = ctx.enter_context(tc.tile_pool(name="psgt", bufs=2, space="PSUM"))
        ps_f = ctx.enter_context(tc.tile_pool(name="psf", bufs=2, space="PSUM"))
        ps_sm = ctx.enter_context(tc.tile_pool(name="pssm", bufs=2, space="PSUM"))

        nc.gpsimd.load_library(library_config.mlp)

        # ------------- resident loads -------------
        srcidx_sb = persist.tile([128, NBLK * 8], I16)
        nc.sync.dma_start(srcidx_sb[:], inp["srcidx"][:])
        p0_sb = persist.tile([128, NBLK * 16], F32)
        nc.sync.dma_start(p0_sb[:], inp["p0"][:])
        p0rep_sb = persist.tile([128, NBLK * 96], BF16)
        nc.sync.dma_start(p0rep_sb[:], inp["p0rep"][:])
        ident_sb = persist.tile([128, 128], BF16)
        nc.sync.dma_start(ident_sb[:], inp["ident"][:])
        dataT_sb = persist.tile([16, NLP], F32)
        nc.sync.dma_start(dataT_sb[:], inp["dataT"][:])
        linW_sb = persist.tile([16, 254], F32)
        nc.sync.dma_start(linW_sb[:], inp["linW"][:])
        bias1_sb = persist.tile([128, 254], F32)
        nc.sync.dma_start(bias1_sb[:], inp["bias1"][:])
        cloc_sb = persist.tile([128, NSLAB, 2], F32)
        nc.sync.dma_start(cloc_sb[:],
                          inp["coords_loc"][:].rearrange("(s p) c -> p s c", p=128))
        cfac_sb = persist.tile([128, NSLAB, 1], F32)
        nc.sync.dma_start(cfac_sb[:],
                          inp["cfac"][:].rearrange("(s p) c -> p s c", p=128))
        wsc_sb = {}
        for i in range(1, 5):
            ktf = FDIM[i - 1] // 128
            t = persist.tile([128, ktf * 12], BF16, tag=f"wsc{i}", name=f"wsc{i}_sb")
            nc.sync.dma_start(t[:], inp[f"wsc{i}"][:])
            wsc_sb[i] = t
        biasg_sb = {}
        for i in range(1, 4):
            C = GAT_DIMS[i - 1][1]
            t = persist.tile([128, C], F32, tag=f"biasg{i}", name=f"biasg{i}_sb")
            nc.sync.dma_start(t[:], inp[f"biasg{i}"][:])
            biasg_sb[i] = t

        # per-stage state
        SDall = persist.tile([128, NSLAB, 6], F32)
        SDblk = persist.tile([16, NBLK, 6], F32)
        CSTK = persist.tile([128, NSLAB, 8], F32)
        OUTC = persist.tile([128, NSLAB, 2], F32)

        # DRAM tables
        sd_dram = dram.tile([NLP, 6], F32, name="sd_dram")
        agin = {}
        feat = {}
        for s in range(1, 5):
            agin[s] = dram.tile([NLP, ROWB[s - 1]], BF16, tag=f"agin{s}",
                                name=f"agin{s}")
            feat[s] = dram.tile([NCORES * NLP, ROWB[s - 1]], BF16, tag=f"feat{s}",
                                name=f"feat{s}")

        # =========================================================
        def selu_into(dst_ap, psum_ap, bias_ap, C):
            """dst = selu(psum[:, :C] + bias)  (dst may be bf16)"""
            t1 = e_pool.tile([128, C], F32, tag="selu_t1")
            nc.vector.tensor_add(t1[:], psum_ap, bias_ap)
            mn = e_pool.tile([128, C], F32, tag="selu_mn")
            nc.vector.tensor_scalar_min(mn[:], t1[:], 0.0)
            ex = e_pool.tile([128, C], F32, tag="selu_ex")
            nc.scalar.activation(ex[:], mn[:], AF.Exp)
            em = e_pool.tile([128, C], F32, tag="selu_em")
            nc.vector.tensor_scalar(em[:], ex[:], SELU_L * SELU_A,
                                    -SELU_L * SELU_A,
                                    mybir.AluOpType.mult, mybir.AluOpType.add)
            rp = e_pool.tile([128, C], F32, tag="selu_rp")
            nc.scalar.activation(rp[:], t1[:], AF.Relu, scale=SELU_L)
            nc.vector.tensor_add(dst_ap, em[:], rp[:])

        def produce(stage, s, psum_f):
            """psum_f -> FN (bf16 feat row) for slab s; scores; ship."""
            din_out = FDIM[stage - 1]
            rowlen = ROWB[stage - 1]
            FN = fn_pool.tile([128, rowlen], BF16, tag="FN")
            if stage == 1:
                nc.vector.tensor_copy(FN[:, 0:2], cloc_sb[:, s, :])
                nc.vector.tensor_copy(CSTK[:, s, 6:8], cloc_sb[:, s, :])
                selu_into(FN[:, 2:256], psum_f[:, 0:254], bias1_sb[:], 254)
            else:
                C = GAT_DIMS[stage - 2][1]
                cnode = CSTK[:, s, 10 - 2 * stage:12 - 2 * stage]
                tcf = small.tile([128, 2], F32, tag="coord_t")
                nc.vector.tensor_scalar(tcf[:], cnode, cfac_sb[:, s, :], None,
                                        mybir.AluOpType.mult)
                cnw = small.tile([128, 2], F32, tag="cnw")
                nc.vector.tensor_add(cnw[:], psum_f[:, C:C + 2], tcf[:])
                nc.vector.tensor_copy(FN[:, 0:2], cnw[:])
                nc.vector.tensor_copy(CSTK[:, s, 8 - 2 * stage:10 - 2 * stage],
                                      cnw[:])
                nstk = 2 * (stage - 1)
                nc.vector.tensor_copy(FN[:, 2:2 + nstk],
                                      CSTK[:, s, 10 - 2 * stage:8])
                selu_into(FN[:, 2 + nstk:2 + nstk + C], psum_f[:, 0:C],
                          biasg_sb[stage - 1][:], C)
            nc.vector.memset(FN[:, din_out + 6:rowlen], 0.0)
            # scores for GAT layer `stage`
            ktf = din_out // 128
            psum_s = ps_sm.tile([128, 12], F32, tag="pssmall")
            for kt in range(ktf):
                pt = ps_sm.tile([128, 128], BF16, tag="pssmall")
                nc.tensor.transpose(pt[:], FN[:, 128 * kt:128 * (kt + 1)], ident_sb[:])
                fnt = fnt_pool.tile([128, 128], BF16, tag="fnt")
                nc.vector.tensor_copy(fnt[:], pt[:])
                nc.tensor.matmul(psum_s[:], fnt[:],
                                 wsc_sb[stage][:, 12 * kt:12 * (kt + 1)],
                                 start=(kt == 0), stop=(kt == ktf - 1))
            nc.vector.tensor_copy(FN[:, din_out:din_out + 6], psum_s[:, 0:6])
            nc.vector.tensor_copy(SDall[:, s, :], psum_s[:, 6:12])
            nc.sync.dma_start(agin[stage][128 * s:128 * (s + 1), :], FN[:])

        def sd_reshape():
            nc.sync.dma_start(
                sd_dram[:].rearrange("(s p) h -> p s h", p=128), SDall[:])
            nc.sync.dma_start(
                SDblk[:], sd_dram[:].rearrange("(c d) h -> d c h", d=16))

        # =========================================================
        def ag_piece(stage, q):
            # fifth q: local rows [256q, 256q+256) -> feat rows [2048q, +2048)
            nc.gpsimd.collective_compute(
                "AllGather", mybir.AluOpType.bypass, replica_groups=rg,
                ins=[agin[stage][256 * q:256 * (q + 1), :].opt()],
                outs=[feat[stage][2048 * q:2048 * (q + 1), :].opt()])

        # STAGE 1: feat1 from data
        for s in range(NSLAB):
            psum_f = ps_f.tile([128, 254], F32, tag="psum_f")
            nc.tensor.matmul(psum_f[:], dataT_sb[0:10, 128 * s:128 * (s + 1)],
                             linW_sb[0:10, :], start=True, stop=True)
            produce(1, s, psum_f)
            if s % 2 == 1:
                ag_piece(1, s // 2)
        sd_reshape()

        # =========================================================
        # STAGES 2..5: GAT layers 1..4
        for stage in range(2, 6):
            g = stage - 1
            din, C = GAT_DIMS[g - 1]
            kt = din // 128
            CP = 2 if g == 4 else C + 2
            rowlen = ROWB[g - 1]
            ftab = feat[g]

            wp_t = wp_pool.tile([128, kt * 6 * CP], BF16, tag="wp")
            nc.sync.dma_start(wp_t[:], inp[f"wp{g}"][:])

            for s in range(NSLAB):
                # ---- gathers ----
                Fg = fg_pool.tile([128, 8, rowlen], BF16, tag="Fg")
                nc.gpsimd.dma_gather(Fg[:], ftab[:],
                                     srcidx_sb[:, 64 * s:64 * (s + 1)],
                                     1024, 1024, rowlen)
                p0t_t = p0t_pool.tile([16, 8, 128], F32, tag="p0t_t")
                nc.sync.dma_start(
                    p0t_t[:], inp["p0t"][:, 1024 * s:1024 * (s + 1)]
                    .rearrange("d (b e) -> d b e", b=8))

                # ---- edge phase ----
                pbc = ps_sm.tile([128, 8, 6], F32, tag="pbc")
                for b in range(8):
                    c = 8 * s + b
                    nc.tensor.matmul(pbc[:, b, :], p0t_t[:, b, :], SDblk[:, c, :],
                                     start=True, stop=True)
                E_sl = e_pool.tile([128, 8, 6], F32, tag="E_sl")
                nc.vector.tensor_add(E_sl[:], Fg[:, :, din:din + 6], pbc[:])
                t_lr = e_pool.tile([128, 8, 6], F32, tag="t_lr")
                nc.vector.tensor_scalar_mul(t_lr[:], E_sl[:], 0.2)
                r_lr = e_pool.tile([128, 8, 6], F32, tag="r_lr")
                nc.scalar.activation(r_lr[:], E_sl[:], AF.Relu, scale=0.8)
                E2 = e_pool.tile([128, 8, 6], F32, tag="E2")
                nc.vector.tensor_add(E2[:], t_lr[:], r_lr[:])
                EX = e_pool.tile([128, 8, 6], F32, tag="EX")
                nc.scalar.activation(EX[:], E2[:], AF.Exp)

                pdn = ps_sm.tile([16, 8, 6], F32, tag="pbc")
                for b in range(8):
                    c = 8 * s + b
                    nc.tensor.matmul(pdn[:, b, :], p0_sb[:, 16 * c:16 * (c + 1)],
                                     EX[:, b, :], start=True, stop=True)
                rd = e_pool.tile([16, 8, 6], F32, tag="rd")
                dple = e_pool.tile([16, 8, 6], F32, tag="dple")
                nc.vector.tensor_scalar_add(dple[:], pdn[:], 1e-16)
                nc.vector.reciprocal(rd[:], dple[:])
                prd = ps_sm.tile([128, 8, 6], F32, tag="pbc")
                for b in range(8):
                    nc.tensor.matmul(prd[:, b, :], p0t_t[:, b, :], rd[:, b, :],
                                     start=True, stop=True)
                A_sl = e_pool.tile([128, 8, 6], BF16, tag="A_sl")
                nc.vector.tensor_mul(A_sl[:], EX[:], prd[:])

                M_sl = m_pool.tile([128, 8, 96], BF16, tag="M_sl")
                nc.vector.tensor_mul(
                    M_sl[:].rearrange("p b (h d) -> p b h d", h=6),
                    p0rep_sb[:, 96 * 8 * s:96 * 8 * (s + 1)]
                    .rearrange("p (b h d) -> p b h d", b=8, h=6),
                    A_sl[:].unsqueeze(3).broadcast_to([128, 8, 6, 16]))

                # ---- scatter: Gt cols [ds][h*128 + b*16 + dl] ----
                Gt = gt_pool.tile([128, kt, 768], BF16, tag="Gt")
                for b in range(8):
                    pgt = ps_gt.tile([128, kt * 96], F32, tag="pgt")
                    for ds in range(kt):
                        nc.tensor.matmul(pgt[:, 96 * ds:96 * (ds + 1)],
                                         Fg[:, b, 128 * ds:128 * (ds + 1)],
                                         M_sl[:, b, :], start=True, stop=True)
                    eng_copy = (nc.vector.tensor_copy if b % 2 == 0
                                else nc.scalar.copy)
                    eng_copy(
                        Gt[:].rearrange("p d (h2 b2 e) -> p d h2 b2 e",
                                        h2=6, b2=8)[:, :, :, b, :],
                        pgt[:].rearrange("p (d h2 e) -> p d h2 e", d=kt, h2=6))

                # ---- feature matmul ----
                psum_f = ps_f.tile([128, CP], F32, tag="psum_f")
                nmm = kt * 6
                i_mm = 0
                for ds in range(kt):
                    for h in range(6):
                        nc.tensor.matmul(psum_f[:], Gt[:, ds, 128 * h:128 * (h + 1)],
                                         wp_t[:, (ds * 6 + h) * CP:(ds * 6 + h + 1) * CP],
                                         start=(i_mm == 0), stop=(i_mm == nmm - 1))
                        i_mm += 1

                # ---- postprocess ----
                if stage < 5:
                    produce(stage, s, psum_f)
                    if s % 2 == 1:
                        ag_piece(stage, s // 2)
                else:
                    cnode = CSTK[:, s, 2:4]
                    tcf = small.tile([128, 2], F32, tag="coord_t")
                    nc.vector.tensor_scalar(tcf[:], cnode, cfac_sb[:, s, :], None,
                                            mybir.AluOpType.mult)
                    nc.vector.tensor_add(OUTC[:, s, :], psum_f[:, 0:2], tcf[:])

            if stage < 5:
                sd_reshape()

        nc.sync.dma_start(out_t[:].rearrange("(s p) c -> p s c", p=128), OUTC[:])

    nc.compile()
    return nc


# ================================================================
def _host_prep(inputs):
    data = np.asarray(inputs["data"], np.float32)
    eidx = np.asarray(inputs["edge_idx"])
    src_a, dst_a = eidx[0].astype(np.int64), eidx[1].astype(np.int64)
    order = np.argsort(dst_a, kind="stable")
    src_s, dst_s = src_a[order], dst_a[order]
    indeg = np.bincount(dst_a, minlength=N)

    shared = {}
    linW = np.zeros((16, 254), np.float32)
    linW[0:10] = np.asarray(inputs["lin_W"], np.float32)
    shared["linW"] = linW
    shared["bias1"] = np.tile(np.asarray(inputs["lin_b"], np.float32)[None, :], (128, 1))
    shared["ident"] = np.eye(128, dtype=BF)
    for i in range(1, 5):
        din, C = GAT_DIMS[i - 1]
        kt = din // 128
        CP = 2 if i == 4 else C + 2
        W = np.asarray(inputs[f"W{i}"], np.float32).reshape(din, H, C)
        wp = np.zeros((din, H, CP), np.float32)
        if i < 4:
            wp[:, :, :C] = W / H
            shared[f"biasg{i}"] = np.tile(np.asarray(inputs[f"b{i}"], np.float32)[None, :], (128, 1))
        wp[0, :, CP - 2] = 1.0 / H
        wp[1, :, CP - 1] = 1.0 / H
        wp_h = np.zeros((128, kt * H * CP), np.float32)
        for ds in range(kt):
            wp_h[:, ds * H * CP:(ds + 1) * H * CP] = \
                wp[ds * 128:(ds + 1) * 128].reshape(128, H * CP)
        shared[f"wp{i}"] = wp_h.astype(BF)
        a_s = np.asarray(inputs[f"as{i}"], np.float32)
        a_d = np.asarray(inputs[f"ad{i}"], np.float32)
        ws = np.einsum("dhc,hc->dh", W, a_s)
        wd = np.einsum("dhc,hc->dh", W, a_d)
        wsc = np.concatenate([ws, wd], 1)
        ktf = FDIM[i - 1] // 128
        wsc_h = np.zeros((128, ktf * 12), np.float32)
        for ds in range(ktf):
            wsc_h[:, ds * 12:(ds + 1) * 12] = wsc[ds * 128:(ds + 1) * 128]
        shared[f"wsc{i}"] = wsc_h.astype(BF)

    in_maps = []
    for r in range(NCORES):
        m = dict(shared)
        lo, hi = NL * r, NL * (r + 1)
        dT = np.zeros((16, NLP), np.float32)
        dT[0:10, 0:NL] = data[lo:hi].T
        m["dataT"] = dT
        cl = np.zeros((NLP, 2), np.float32)
        cl[0:NL] = data[lo:hi, 0:2]
        m["coords_loc"] = cl
        cf = np.ones((NLP, 1), np.float32)
        cf[0:NL, 0] = (indeg[lo:hi] == 0).astype(np.float32)
        m["cfac"] = cf

        sel = (dst_s >= lo) & (dst_s < hi)
        es, ed = src_s[sel], dst_s[sel] - lo
        p0 = np.zeros((128, NBLK * 16), np.float32)
        p0t = np.zeros((16, NBLK * 128), np.float32)
        p0rep = np.zeros((128, NBLK * 96), np.float32)
        sidx = np.zeros((128, NBLK * 8), np.int16)
        blk = ed // 16
        for c in range(NBLK):
            emask = blk == c
            k = int(emask.sum())
            assert k <= CAP, f"block overflow core {r} blk {c}: {k}"
            if k == 0:
                continue
            srcs = es[emask]
            lds = ed[emask].astype(np.int64)
            dls = lds % 16
            p0c = np.zeros((128, 16), np.float32)
            p0c[np.arange(k), dls] = 1.0
            p0[:, 16 * c:16 * (c + 1)] = p0c
            p0t[:, 128 * c:128 * (c + 1)] = p0c.T
            p0rep[:, 96 * c:96 * (c + 1)] = np.tile(p0c, (1, 6))
            rr = srcs // NL
            ii = srcs % NL
            agrow = (ii // 256) * 2048 + rr * 256 + (ii % 256)
            fulls = np.zeros(128, np.int64)
            fulls[:k] = agrow
            s_i, b_i = c // 8, c % 8
            ws_ = sidx[:, 64 * s_i:64 * (s_i + 1)]
            for e_i in range(128):
                gk = 128 * b_i + e_i
                ws_[gk % 16, gk // 16] = fulls[e_i]
        for s_i in range(NSLAB):
            w = sidx[:, 64 * s_i:64 * (s_i + 1)]
            w[16:] = np.tile(w[:16], (7, 1))
        m["p0"] = p0
        m["p0t"] = p0t
        m["p0rep"] = p0rep.astype(BF)
        m["srcidx"] = sidx
        in_maps.append(m)
    return in_maps


_NC_CACHE = None


def kernel(**inputs):
    global _NC_CACHE
    in_maps = _host_prep(inputs)
    if _NC_CACHE is None:
        _NC_CACHE = _build_nc()
    res = run_bass_kernel_spmd(_NC_CACHE, in_maps, core_ids=list(range(NCORES)))
    out = np.zeros((N, 2), np.float32)
    for r in range(NCORES):
        out[NL * r:NL * (r + 1)] = res.results[r]["out"][:NL]
    return out


# revision 15
# speedup vs baseline: 1.0978x; 1.0978x over previous
"""DeformGAT (4-layer) Trainium2 kernel — 8 NeuronCores SPMD.

Sharding: nodes in 8 contiguous blocks of 1250 (padded to 1280); edges are
assigned to their dst node's core (edges pre-sorted by dst on host). Weights
replicated. Per layer each core gathers src rows of the replicated bf16
feature table (dma_gather), computes per-edge softmax (score broadcasts via
dst-indexed gathers + one-hot segment-sum matmuls), aggregates with bf16
scatter matmuls, applies the per-head output transform (head-mean, bias and
coordinate displacement folded into an augmented weight matrix), then
AllGathers its produced rows so every core again holds the full table.
"""
import numpy as np
import ml_dtypes
from contextlib import ExitStack

import concourse.bacc as bacc
import concourse.bass as bass
import concourse.tile as tile
import concourse.mybir as mybir
from concourse import library_config
from concourse.bass_utils import run_bass_kernel_spmd

F32 = mybir.dt.float32
BF16 = mybir.dt.bfloat16
I16 = mybir.dt.int16
AF = mybir.ActivationFunctionType
BF = ml_dtypes.bfloat16

NCORES = 8
N = 10000
E = 60000
H = 6
NL = 1250          # real nodes per core
NLP = 1280         # padded nodes per core
NBLK = 80          # dst blocks of 16 per core
NSLAB = 10         # slabs of 128 dst nodes (8 blocks)
CAP = 128          # edge capacity per block (= chunk)

# GAT layer dims (din, C). Stage s (2..5) runs GAT s-1.
GAT_DIMS = [(256, 508), (512, 250), (256, 120), (128, 20)]
FDIM = [256, 512, 256, 128]          # feat_s dim produced by stage s
ROWB = [384, 640, 384, 256]          # bf16 row elems (din + 6 scores, %128)

SELU_L = 1.0507009873554805
SELU_A = 1.6732632423543772


def _build_nc():
    nc = bacc.Bacc("TRN2", target_bir_lowering=False, debug=False,
                   num_devices=NCORES)
    # ---------------- inputs ----------------
    inp = {}
    inp["dataT"] = nc.dram_tensor("dataT", [16, NLP], F32, kind="ExternalInput")
    inp["coords_loc"] = nc.dram_tensor("coords_loc", [NLP, 2], F32, kind="ExternalInput")
    inp["cfac"] = nc.dram_tensor("cfac", [NLP, 1], F32, kind="ExternalInput")
    inp["srcidx"] = nc.dram_tensor("srcidx", [128, NBLK * 8], I16, kind="ExternalInput")
    inp["p0"] = nc.dram_tensor("p0", [128, NBLK * 16], F32, kind="ExternalInput")
    inp["p0t"] = nc.dram_tensor("p0t", [16, NBLK * 128], F32, kind="ExternalInput")
    inp["p0rep"] = nc.dram_tensor("p0rep", [128, NBLK * 96], BF16, kind="ExternalInput")
    inp["ident"] = nc.dram_tensor("ident", [128, 128], BF16, kind="ExternalInput")
    inp["linW"] = nc.dram_tensor("linW", [16, 254], F32, kind="ExternalInput")
    inp["bias1"] = nc.dram_tensor("bias1", [128, 254], F32, kind="ExternalInput")
    for i in range(1, 5):
        din, C = GAT_DIMS[i - 1]
        kt = din // 128
        CP = 2 if i == 4 else C + 2
        inp[f"wp{i}"] = nc.dram_tensor(f"wp{i}", [128, kt * 6 * CP], BF16, kind="ExternalInput")
        if i < 4:
            inp[f"biasg{i}"] = nc.dram_tensor(f"biasg{i}", [128, C], F32, kind="ExternalInput")
        ktf = FDIM[i - 1] // 128
        inp[f"wsc{i}"] = nc.dram_tensor(f"wsc{i}", [128, ktf * 12], BF16, kind="ExternalInput")
    out_t = nc.dram_tensor("out", [NLP, 2], F32, kind="ExternalOutput")

    rg = [list(range(NCORES))]

    with tile.TileContext(nc) as tc, ExitStack() as ctx:
        persist = ctx.enter_context(tc.tile_pool(name="persist", bufs=1))
        dram = ctx.enter_context(tc.tile_pool(name="dram", bufs=1, space="DRAM"))
        fg_pool = ctx.enter_context(tc.tile_pool(name="fg", bufs=3))
        gt_pool = ctx.enter_context(tc.tile_pool(name="gt", bufs=2))
        m_pool = ctx.enter_context(tc.tile_pool(name="m", bufs=2))
        e_pool = ctx.enter_context(tc.tile_pool(name="ep", bufs=2))
        fn_pool = ctx.enter_context(tc.tile_pool(name="fn", bufs=2))
        fnt_pool = ctx.enter_context(tc.tile_pool(name="fnt", bufs=2))
        wp_pool = ctx.enter_context(tc.tile_pool(name="wp", bufs=2))
        p0t_pool = ctx.enter_context(tc.tile_pool(name="p0tp", bufs=2))
        small = ctx.enter_context(tc.tile_pool(name="small", bufs=3))
        ps_gt = ctx.enter_context(tc.tile_pool(name="psgt", bufs=2, space="PSUM"))
        ps_f = ctx.enter_context(tc.tile_pool(name="psf", bufs=2, space="PSUM"))
        ps_sm = ctx.enter_context(tc.tile_pool(name="pssm", bufs=2, space="PSUM"))

        nc.gpsimd.load_library(library_config.mlp)

        # ------------- resident loads -------------
        srcidx_sb = persist.tile([128, NBLK * 8], I16)
        nc.sync.dma_start(srcidx_sb[:], inp["srcidx"][:])
        p0_sb = persist.tile([128, NBLK * 16], F32)
        nc.sync.dma_start(p0_sb[:], inp["p0"][:])
        p0rep_sb = persist.tile([128, NBLK * 96], BF16)
        nc.sync.dma_start(p0rep_sb[:], inp["p0rep"][:])
        ident_sb = persist.tile([128, 128], BF16)
        nc.sync.dma_start(ident_sb[:], inp["ident"][:])
        dataT_sb = persist.tile([16, NLP], F32)
        nc.sync.dma_start(dataT_sb[:], inp["dataT"][:])
        linW_sb = persist.tile([16, 254], F32)
        nc.sync.dma_start(linW_sb[:], inp["linW"][:])
        bias1_sb = persist.tile([128, 254], F32)
        nc.sync.dma_start(bias1_sb[:], inp["bias1"][:])
        cloc_sb = persist.tile([128, NSLAB, 2], F32)
        nc.sync.dma_start(cloc_sb[:],
                          inp["coords_loc"][:].rearrange("(s p) c -> p s c", p=128))
        cfac_sb = persist.tile([128, NSLAB, 1], F32)
        nc.sync.dma_start(cfac_sb[:],
                          inp["cfac"][:].rearrange("(s p) c -> p s c", p=128))
        wsc_sb = {}
        for i in range(1, 5):
            ktf = FDIM[i - 1] // 128
            t = persist.tile([128, ktf * 12], BF16, tag=f"wsc{i}", name=f"wsc{i}_sb")
            nc.sync.dma_start(t[:], inp[f"wsc{i}"][:])
            wsc_sb[i] = t
        biasg_sb = {}
        for i in range(1, 4):
            C = GAT_DIMS[i - 1][1]
            t = persist.tile([128, C], F32, tag=f"biasg{i}", name=f"biasg{i}_sb")
            nc.sync.dma_start(t[:], inp[f"biasg{i}"][:])
            biasg_sb[i] = t

        # per-stage state
        SDall = persist.tile([128, NSLAB, 6], F32)
        SDblk = persist.tile([16, NBLK, 6], F32)
        CSTK = persist.tile([128, NSLAB, 8], F32)
        OUTC = persist.tile([128, NSLAB, 2], F32)

        # DRAM tables
        sd_dram = dram.tile([NLP, 6], F32, name="sd_dram")
        agin = {}
        feat = {}
        for s in range(1, 5):
            agin[s] = dram.tile([NLP, ROWB[s - 1]], BF16, tag=f"agin{s}",
                                name=f"agin{s}")
            feat[s] = dram.tile([NCORES * NLP, ROWB[s - 1]], BF16, tag=f"feat{s}",
                                name=f"feat{s}")

        # =========================================================
        def selu_into(dst_ap, psum_ap, bias_ap, C):
            """dst = selu(psum[:, :C] + bias)  (dst may be bf16)"""
            t1 = e_pool.tile([128, C], F32, tag="selu_t1")
            nc.vector.tensor_add(t1[:], psum_ap, bias_ap)
            mn = e_pool.tile([128, C], F32, tag="selu_mn")
            nc.vector.tensor_scalar_min(mn[:], t1[:], 0.0)
            ex = e_pool.tile([128, C], F32, tag="selu_ex")
            nc.scalar.activation(ex[:], mn[:], AF.Exp)
            em = e_pool.tile([128, C], F32, tag="selu_em")
            nc.vector.tensor_scalar(em[:], ex[:], SELU_L * SELU_A,
                                    -SELU_L * SELU_A,
                                    mybir.AluOpType.mult, mybir.AluOpType.add)
            rp = e_pool.tile([128, C], F32, tag="selu_rp")
            nc.scalar.activation(rp[:], t1[:], AF.Relu, scale=SELU_L)
            nc.vector.tensor_add(dst_ap, em[:], rp[:])

        def produce(stage, s, psum_f):
            """psum_f -> FN (bf16 feat row) for slab s; scores; ship."""
            din_out = FDIM[stage - 1]
            rowlen = ROWB[stage - 1]
            FN = fn_pool.tile([128, rowlen], BF16, tag="FN")
            if stage == 1:
                nc.vector.tensor_copy(FN[:, 0:2], cloc_sb[:, s, :])
                nc.vector.tensor_copy(CSTK[:, s, 6:8], cloc_sb[:, s, :])
                selu_into(FN[:, 2:256], psum_f[:, 0:254], bias1_sb[:], 254)
            else:
                C = GAT_DIMS[stage - 2][1]
                cnode = CSTK[:, s, 10 - 2 * stage:12 - 2 * stage]
                tcf = small.tile([128, 2], F32, tag="coord_t")
                nc.vector.tensor_scalar(tcf[:], cnode, cfac_sb[:, s, :], None,
                                        mybir.AluOpType.mult)
                cnw = small.tile([128, 2], F32, tag="cnw")
                nc.vector.tensor_add(cnw[:], psum_f[:, C:C + 2], tcf[:])
                nc.vector.tensor_copy(FN[:, 0:2], cnw[:])
                nc.vector.tensor_copy(CSTK[:, s, 8 - 2 * stage:10 - 2 * stage],
                                      cnw[:])
                nstk = 2 * (stage - 1)
                nc.vector.tensor_copy(FN[:, 2:2 + nstk],
                                      CSTK[:, s, 10 - 2 * stage:8])
                selu_into(FN[:, 2 + nstk:2 + nstk + C], psum_f[:, 0:C],
                          biasg_sb[stage - 1][:], C)
            nc.vector.memset(FN[:, din_out + 6:rowlen], 0.0)
            # scores for GAT layer `stage`
            ktf = din_out // 128
            psum_s = ps_sm.tile([128, 12], F32, tag="pssmall")
            for kt in range(ktf):
                pt = ps_sm.tile([128, 128], BF16, tag="pssmall")
                nc.tensor.transpose(pt[:], FN[:, 128 * kt:128 * (kt + 1)], ident_sb[:])
                fnt = fnt_pool.tile([128, 128], BF16, tag="fnt")
                nc.vector.tensor_copy(fnt[:], pt[:])
                nc.tensor.matmul(psum_s[:], fnt[:],
                                 wsc_sb[stage][:, 12 * kt:12 * (kt + 1)],
                                 start=(kt == 0), stop=(kt == ktf - 1))
            nc.vector.tensor_copy(FN[:, din_out:din_out + 6], psum_s[:, 0:6])
            nc.vector.tensor_copy(SDall[:, s, :], psum_s[:, 6:12])
            nc.sync.dma_start(agin[stage][128 * s:128 * (s + 1), :], FN[:])

        def sd_reshape():
            nc.sync.dma_start(
                sd_dram[:].rearrange("(s p) h -> p s h", p=128), SDall[:])
            nc.sync.dma_start(
                SDblk[:], sd_dram[:].rearrange("(c d) h -> d c h", d=16))

        # =========================================================
        def ag_half(stage, half):
            lo, hi = (0, 640) if half == 0 else (640, NLP)
            fl = NCORES * 640 * half
            fh = fl + NCORES * 640
            nc.gpsimd.collective_compute(
                "AllGather", mybir.AluOpType.bypass, replica_groups=rg,
                ins=[agin[stage][lo:hi, :].opt()],
                outs=[feat[stage][fl:fh, :].opt()])

        # STAGE 1: feat1 from data
        for s in range(NSLAB):
            psum_f = ps_f.tile([128, 254], F32, tag="psum_f")
            nc.tensor.matmul(psum_f[:], dataT_sb[0:10, 128 * s:128 * (s + 1)],
                             linW_sb[0:10, :], start=True, stop=True)
            produce(1, s, psum_f)
            if s == 4:
                ag_half(1, 0)
        ag_half(1, 1)
        sd_reshape()

        # =========================================================
        # STAGES 2..5: GAT layers 1..4
        for stage in range(2, 6):
            g = stage - 1
            din, C = GAT_DIMS[g - 1]
            kt = din // 128
            CP = 2 if g == 4 else C + 2
            rowlen = ROWB[g - 1]
            ftab = feat[g]

            wp_t = wp_pool.tile([128, kt * 6 * CP], BF16, tag="wp")
            nc.sync.dma_start(wp_t[:], inp[f"wp{g}"][:])

            for s in range(NSLAB):
                # ---- gathers ----
                Fg = fg_pool.tile([128, 8, rowlen], BF16, tag="Fg")
                nc.gpsimd.dma_gather(Fg[:], ftab[:],
                                     srcidx_sb[:, 64 * s:64 * (s + 1)],
                                     1024, 1024, rowlen)
                p0t_t = p0t_pool.tile([16, 8, 128], F32, tag="p0t_t")
                nc.sync.dma_start(
                    p0t_t[:], inp["p0t"][:, 1024 * s:1024 * (s + 1)]
                    .rearrange("d (b e) -> d b e", b=8))

                # ---- edge phase ----
                pbc = ps_sm.tile([128, 8, 6], F32, tag="pbc")
                for b in range(8):
                    c = 8 * s + b
                    nc.tensor.matmul(pbc[:, b, :], p0t_t[:, b, :], SDblk[:, c, :],
                                     start=True, stop=True)
                E_sl = e_pool.tile([128, 8, 6], F32, tag="E_sl")
                nc.vector.tensor_add(E_sl[:], Fg[:, :, din:din + 6], pbc[:])
                t_lr = e_pool.tile([128, 8, 6], F32, tag="t_lr")
                nc.vector.tensor_scalar_mul(t_lr[:], E_sl[:], 0.2)
                r_lr = e_pool.tile([128, 8, 6], F32, tag="r_lr")
                nc.scalar.activation(r_lr[:], E_sl[:], AF.Relu, scale=0.8)
                E2 = e_pool.tile([128, 8, 6], F32, tag="E2")
                nc.vector.tensor_add(E2[:], t_lr[:], r_lr[:])
                EX = e_pool.tile([128, 8, 6], F32, tag="EX")
                nc.scalar.activation(EX[:], E2[:], AF.Exp)

                pdn = ps_sm.tile([16, 8, 6], F32, tag="pbc")
                for b in range(8):
                    c = 8 * s + b
                    nc.tensor.matmul(pdn[:, b, :], p0_sb[:, 16 * c:16 * (c + 1)],
                                     EX[:, b, :], start=True, stop=True)
                rd = e_pool.tile([16, 8, 6], F32, tag="rd")
                dple = e_pool.tile([16, 8, 6], F32, tag="dple")
                nc.vector.tensor_scalar_add(dple[:], pdn[:], 1e-16)
                nc.vector.reciprocal(rd[:], dple[:])
                prd = ps_sm.tile([128, 8, 6], F32, tag="pbc")
                for b in range(8):
                    nc.tensor.matmul(prd[:, b, :], p0t_t[:, b, :], rd[:, b, :],
                                     start=True, stop=True)
                A_sl = e_pool.tile([128, 8, 6], BF16, tag="A_sl")
                nc.vector.tensor_mul(A_sl[:], EX[:], prd[:])

                M_sl = m_pool.tile([128, 8, 96], BF16, tag="M_sl")
                nc.vector.tensor_mul(
                    M_sl[:].rearrange("p b (h d) -> p b h d", h=6),
                    p0rep_sb[:, 96 * 8 * s:96 * 8 * (s + 1)]
                    .rearrange("p (b h d) -> p b h d", b=8, h=6),
                    A_sl[:].unsqueeze(3).broadcast_to([128, 8, 6, 16]))

                # ---- scatter: Gt cols [ds][h*128 + b*16 + dl] ----
                Gt = gt_pool.tile([128, kt, 768], BF16, tag="Gt")
                for b in range(8):
                    pgt = ps_gt.tile([128, kt * 96], F32, tag="pgt")
                    for ds in range(kt):
                        nc.tensor.matmul(pgt[:, 96 * ds:96 * (ds + 1)],
                                         Fg[:, b, 128 * ds:128 * (ds + 1)],
                                         M_sl[:, b, :], start=True, stop=True)
                    eng_copy = (nc.vector.tensor_copy if b % 2 == 0
                                else nc.scalar.copy)
                    eng_copy(
                        Gt[:].rearrange("p d (h2 b2 e) -> p d h2 b2 e",
                                        h2=6, b2=8)[:, :, :, b, :],
                        pgt[:].rearrange("p (d h2 e) -> p d h2 e", d=kt, h2=6))

                # ---- feature matmul ----
                psum_f = ps_f.tile([128, CP], F32, tag="psum_f")
                nmm = kt * 6
                i_mm = 0
                for ds in range(kt):
                    for h in range(6):
                        nc.tensor.matmul(psum_f[:], Gt[:, ds, 128 * h:128 * (h + 1)],
                                         wp_t[:, (ds * 6 + h) * CP:(ds * 6 + h + 1) * CP],
                                         start=(i_mm == 0), stop=(i_mm == nmm - 1))
                        i_mm += 1

                # ---- postprocess ----
                if stage < 5:
                    produce(stage, s, psum_f)
                    if s == 4:
                        ag_half(stage, 0)
                else:
                    cnode = CSTK[:, s, 2:4]
                    tcf = small.tile([128, 2], F32, tag="coord_t")
                    nc.vector.tensor_scalar(tcf[:], cnode, cfac_sb[:, s, :], None,
                                            mybir.AluOpType.mult)
                    nc.vector.tensor_add(OUTC[:, s, :], psum_f[:, 0:2], tcf[:])

            if stage < 5:
                ag_half(stage, 1)
                sd_reshape()

        nc.sync.dma_start(out_t[:].rearrange("(s p) c -> p s c", p=128), OUTC[:])

    nc.compile()
    return nc


# ================================================================
def _host_prep(inputs):
    data = np.asarray(inputs["data"], np.float32)
    eidx = np.asarray(inputs["edge_idx"])
    src_a, dst_a = eidx[0].astype(np.int64), eidx[1].astype(np.int64)
    order = np.argsort(dst_a, kind="stable")
    src_s, dst_s = src_a[order], dst_a[order]
    indeg = np.bincount(dst_a, minlength=N)

    shared = {}
    linW = np.zeros((16, 254), np.float32)
    linW[0:10] = np.asarray(inputs["lin_W"], np.float32)
    shared["linW"] = linW
    shared["bias1"] = np.tile(np.asarray(inputs["lin_b"], np.float32)[None, :], (128, 1))
    shared["ident"] = np.eye(128, dtype=BF)
    for i in range(1, 5):
        din, C = GAT_DIMS[i - 1]
        kt = din // 128
        CP = 2 if i == 4 else C + 2
        W = np.asarray(inputs[f"W{i}"], np.float32).reshape(din, H, C)
        wp = np.zeros((din, H, CP), np.float32)
        if i < 4:
            wp[:, :, :C] = W / H
            shared[f"biasg{i}"] = np.tile(np.asarray(inputs[f"b{i}"], np.float32)[None, :], (128, 1))
        wp[0, :, CP - 2] = 1.0 / H
        wp[1, :, CP - 1] = 1.0 / H
        wp_h = np.zeros((128, kt * H * CP), np.float32)
        for ds in range(kt):
            wp_h[:, ds * H * CP:(ds + 1) * H * CP] = \
                wp[ds * 128:(ds + 1) * 128].reshape(128, H * CP)
        shared[f"wp{i}"] = wp_h.astype(BF)
        a_s = np.asarray(inputs[f"as{i}"], np.float32)
        a_d = np.asarray(inputs[f"ad{i}"], np.float32)
        ws = np.einsum("dhc,hc->dh", W, a_s)
        wd = np.einsum("dhc,hc->dh", W, a_d)
        wsc = np.concatenate([ws, wd], 1)
        ktf = FDIM[i - 1] // 128
        wsc_h = np.zeros((128, ktf * 12), np.float32)
        for ds in range(ktf):
            wsc_h[:, ds * 12:(ds + 1) * 12] = wsc[ds * 128:(ds + 1) * 128]
        shared[f"wsc{i}"] = wsc_h.astype(BF)

    in_maps = []
    for r in range(NCORES):
        m = dict(shared)
        lo, hi = NL * r, NL * (r + 1)
        dT = np.zeros((16, NLP), np.float32)
        dT[0:10, 0:NL] = data[lo:hi].T
        m["dataT"] = dT
        cl = np.zeros((NLP, 2), np.float32)
        cl[0:NL] = data[lo:hi, 0:2]
        m["coords_loc"] = cl
        cf = np.ones((NLP, 1), np.float32)
        cf[0:NL, 0] = (indeg[lo:hi] == 0).astype(np.float32)
        m["cfac"] = cf

        sel = (dst_s >= lo) & (dst_s < hi)
        es, ed = src_s[sel], dst_s[sel] - lo
        p0 = np.zeros((128, NBLK * 16), np.float32)
        p0t = np.zeros((16, NBLK * 128), np.float32)
        p0rep = np.zeros((128, NBLK * 96), np.float32)
        sidx = np.zeros((128, NBLK * 8), np.int16)
        blk = ed // 16
        for c in range(NBLK):
            emask = blk == c
            k = int(emask.sum())
            assert k <= CAP, f"block overflow core {r} blk {c}: {k}"
            if k == 0:
                continue
            srcs = es[emask]
            lds = ed[emask].astype(np.int64)
            dls = lds % 16
            p0c = np.zeros((128, 16), np.float32)
            p0c[np.arange(k), dls] = 1.0
            p0[:, 16 * c:16 * (c + 1)] = p0c
            p0t[:, 128 * c:128 * (c + 1)] = p0c.T
            p0rep[:, 96 * c:96 * (c + 1)] = np.tile(p0c, (1, 6))
            rr = srcs // NL
            ii = srcs % NL
            agrow = np.where(ii < 640, rr * 640 + ii,
                             NCORES * 640 + rr * 640 + (ii - 640))
            fulls = np.zeros(128, np.int64)
            fulls[:k] = agrow
            s_i, b_i = c // 8, c % 8
            ws_ = sidx[:, 64 * s_i:64 * (s_i + 1)]
            for e_i in range(128):
                gk = 128 * b_i + e_i
                ws_[gk % 16, gk // 16] = fulls[e_i]
        for s_i in range(NSLAB):
            w = sidx[:, 64 * s_i:64 * (s_i + 1)]
            w[16:] = np.tile(w[:16], (7, 1))
        m["p0"] = p0
        m["p0t"] = p0t
        m["p0rep"] = p0rep.astype(BF)
        m["srcidx"] = sidx
        in_maps.append(m)
    return in_maps


_NC_CACHE = None


def kernel(**inputs):
    global _NC_CACHE
    in_maps = _host_prep(inputs)
    if _NC_CACHE is None:
        _NC_CACHE = _build_nc()
    res = run_bass_kernel_spmd(_NC_CACHE, in_maps, core_ids=list(range(NCORES)))
    out = np.zeros((N, 2), np.float32)
    for r in range(NCORES):
        out[NL * r:NL * (r + 1)] = res.results[r]["out"][:NL]
    return out
